# revision 70
# baseline (speedup 1.0000x reference)
"""Trainium2 Bass kernel for nn_Euclidean (retrieval_knn).

Computes out[b, c] = -mean_f (x[b, f] - w[c, f])^2 for x [16384, 2048] f32,
w [1000, 2048] f32, via the algebraic expansion

    out = (2/F) * (x @ w.T  -  ||w_c||^2/2)  -  ||x_b||^2 / F

Sharding: data-parallel over the batch dim across 8 NeuronCores; w replicated.

Schedule: software pipeline with prep running LAG rounds ahead; phase G
round r issues [x_load(r+LAG), gemm(r), x_prep(r+LAG)] so the PE stays
HAM-warm (warm transposes ~110ns vs 210ns cold; DR matmuls 215ns).

  - All DMA issues precede any compute instruction.  w loads split
    across BOTH HWDGE rings (w0-3 scalar, w4-7 sync); x loads queue
    behind w4-7 on the sync ring FIFO so w gets full DMA bandwidth
    first.  gemm output stores + the tiny w2 column stores ride the
    GpSimd SWDGE ring so they never head-of-line-block an HWDGE queue.
  - Per-tile prep: fp32 tiles are transposed DIRECTLY on the PE (cayman
    transpose_mode, 2-pass fp32) -- no bf16 cast hop -- in 4 one-bank
    PSUM groups, evacuated to fp8e4 SBUF (w: 1 ScalarE + 3 DVE evacs;
    x: per-phase split).
  - Norms: ScalarE Square with fp32 accum.  w2col buffers are one-per-
    tile (the SWDGE store completion is ~5-8us; fewer buffers made each
    Square WAR-wait a store and serialized the front).  The x0..LAG-1
    preps (norms included) run AFTER the whole w loop: their tiles land
    behind w4..7 on the sync ring, so interleaving them into the w loop
    stalled ready w transposes behind them on the PE FIFO.
  - ||w_c||^2 reaches all partitions via DRAM round-trip: per-tile
    columns -> w2d, then one stride-0-partition broadcast DMA load and
    a ScalarE scale to -w2/F.  Nothing in that chain touches the PE
    queue, so gemm(0) fires as soon as wT/xT0 are resident; only the
    per-round DVE output ADD waits on it.

GEMM: 8 DoubleRow fp8 matmuls per 512-col half accumulate x @ w.T into
one [128,1024] PSUM tile (two banks).  lhsT/rhs are 3D APs [128 ki, 2
plane, free] whose planes are adjacent 128-k chunks (plane stride %16==0
satisfies s3_lw_dual_fp8_restrictions); each instruction contracts 256 k
with its LDWEIGHTS hidden under the previous matmul's stream.  The fp8
quantization noise only touches the (2/F)*x@w.T term (~1e-3 of the
output magnitude).  One ScalarE Identity evacuates both PSUM banks with
scale=2/F and per-partition bias -||x_b||^2/F, then DVE adds the
broadcast -||w_c||^2/F row and the result streams out via SWDGE.

Walrus encodes at most one semaphore wait per LDWEIGHTS/MM struct:
_legalize_waits splits any multi-wait instructions into standalone
EventSemaphore carriers.
"""

import math
import os
import sys

import numpy as np

if "/opt/trn_rl_repo" not in sys.path:
    sys.path.insert(0, "/opt/trn_rl_repo")

N_CORES = 8
B_TOTAL = 16384
F = 2048
C = 1000

_cache = {}
LAST_RESULTS = None


def _legalize_waits(nc):
    """Walrus encodes at most ONE sync-wait per instruction struct, but Tile's
    sem assignment freely attaches several. Split: hoist all but the last wait
    onto standalone EventSemaphore instructions (pure sem-op carriers) placed
    immediately before the over-limit instruction on the same engine queue."""
    import bass_rust
    import concourse.mybir as mybir

    n = 0
    for f in nc.m.functions:
        for bb in f.blocks:
            newlist = []
            for inst in bb.instructions:
                si = inst.sync_info
                if si is not None and len(si.on_wait) > 1:
                    waits = list(si.on_wait)
                    for w in waits[:-1]:
                        ev = mybir.InstEventSemaphore(
                            name=f"waitsplit_{n}", ins=[], outs=[]
                        )
                        ev.engine = inst.engine
                        ev.sync_info = bass_rust.SyncInfo(on_wait=[w], on_update=[])
                        newlist.append(ev)
                        n += 1
                    inst.sync_info = bass_rust.SyncInfo(
                        on_wait=[waits[-1]], on_update=list(si.on_update)
                    )
                newlist.append(inst)
            bb.instructions = newlist
    return n


def _build():
    import concourse.bass as bass
    import concourse.mybir as mybir
    from bass_rust import add_dep_helper
    from concourse.masks import make_identity
    from concourse.tile import TileContext

    P = 128
    KT = F // P                 # 16 contraction chunks of 128
    KD = KT // 2                # 8 DoubleRow plane-pairs of 256
    B = B_TOTAL // N_CORES      # 2048 batch rows per core
    BT = B // P                 # 16 batch chunks
    CP = 1024                   # padded class dim
    CT = CP // P                # 8 class chunks
    KG = 4                      # k-chunks per PSUM transpose group
    LAG = 4                     # x tiles prepped ahead of their gemm
    f8 = mybir.dt.float8e4
    bdt = mybir.dt.bfloat16
    fdt = mybir.dt.float32
    AF = mybir.ActivationFunctionType
    ALU = mybir.AluOpType
    DR = mybir.MatmulPerfMode.DoubleRow

    nc = bass.Bass()
    x = nc.dram_tensor("x", [B, F], fdt, kind="ExternalInput")
    w = nc.dram_tensor("w", [C, F], fdt, kind="ExternalInput")
    out = nc.dram_tensor("out", [B, C], fdt, kind="ExternalOutput")

    with TileContext(nc) as tc:
        with (
            tc.tile_pool(name="consts", bufs=1) as constp,
            tc.tile_pool(name="wstage", bufs=3) as wp,
            tc.tile_pool(name="xstage", bufs=3) as xp,
            tc.tile_pool(name="evac", bufs=3) as ep,
            tc.tile_pool(name="dram", bufs=1, space="DRAM") as dp,
            tc.tile_pool(name="psum", bufs=2, space="PSUM") as pp,
        ):
            # ---- DMA issues first: w on both rings, x behind w on sync ----
            w_f32s = []
            for j in range(CT):
                c0 = j * P
                csz = min(P, C - c0)
                w_f32 = wp.tile([P, F], fdt, tag="w_f32", bufs=8,
                                name=f"w_f32_{j}")
                if csz < P:
                    # the transpose below reads the pad rows; zero them
                    # BEFORE the load so the DMA overwrites the real rows.
                    pad_base = (csz // 32) * 32
                    nc.vector.memset(w_f32[pad_base:P, :], 0.0)
                eng = nc.scalar if j < CT // 2 else nc.sync
                eng.dma_start(out=w_f32[:csz, :], in_=w[c0 : c0 + csz, :])
                w_f32s.append(w_f32)

            def x_load(i):
                x_f32 = xp.tile([P, F], fdt, tag="x_f32", bufs=6,
                                name=f"x_f32_{i}")
                nc.sync.dma_start(out=x_f32[:, :], in_=x[i * P : (i + 1) * P, :])
                return x_f32

            xfs = {}
            for j in range(LAG):
                xfs[j] = x_load(j)

            # ---- constants (emitted after the issues; run during the DMA) --
            # Two identities: w tiles transpose straight from fp32 (the
            # PE has slack in the front and fp32 transpose_mode is a
            # 2-pass ~110ns/chunk affair), x tiles go through a DVE bf16
            # cast and 55ns/chunk bf16 transposes (PE is the bottleneck
            # in phase G).
            identf = constp.tile([P, P], fdt)
            make_identity(nc, identf[:, :])
            identb = constp.tile([P, P], bdt)
            make_identity(nc, identb[:, :])
            pwarm = pp.tile([P, P], fdt, tag="pst", bufs=4)
            nc.tensor.transpose(pwarm[:, :], identf[:, :], identf[:, :])

            # preload both ACT table sets (Square + Identity) off-path
            tw0 = wp.tile([1, 1], fdt, tag="actwarm0")
            tw1 = wp.tile([1, 1], fdt, tag="actwarm1")
            nc.vector.memset(tw0[:, :], 0.0)
            nc.scalar.activation(tw1[:, :], tw0[:, :], AF.Square)
            nc.scalar.activation(tw1[:, :], tw0[:, :], AF.Identity)

            wT = constp.tile([P, KT, CP], f8)     # w^T fp8, resident all kernel
            w2neg = constp.tile([1, CP], fdt)     # -||w_c||^2 / F
            w2row = constp.tile([1, CP], fdt)
            w2d = dp.tile([CP, 1], fdt)
            xTs = [
                constp.tile([P, KT, P], f8, name=f"xT_{i}") for i in range(BT)
            ]
            negx2s = [
                constp.tile([P, 1], fdt, name=f"negx2_{i}") for i in range(BT)
            ]

            dum_pool = {"prev": None}

            def transpose_evac(tile, put_evac, idn, pdt):
                """16 transposes in 4 one-bank PSUM groups + evacs.
                Multi-wait first-transposes are split by _legalize_waits."""
                for kg in range(KT // KG):
                    pst = pp.tile([P, KG * P], pdt, tag="pst", bufs=4)
                    for q in range(KG):
                        k = kg * KG + q
                        nc.tensor.transpose(
                            pst[:, q * P : (q + 1) * P],
                            tile[:, k * P : (k + 1) * P],
                            idn[:, :],
                        )
                    put_evac(kg, pst[:, :].rearrange("p (k c) -> p k c", k=KG))

            def w_setup(j):
                c0 = j * P
                csz = min(P, C - c0)
                w_f32 = w_f32s[j]
                # norm first: it gates the bias-rider fill and reads
                # w_f32 directly, so it never waits on the cast chain.
                wsq = wp.tile([P, F], bdt, tag="wsq", bufs=2)
                # one buffer per tile: the SWDGE store's completion is
                # slow (~5-8us); bufs=2 made each Square WAR-wait the
                # store from two tiles earlier, serializing the front.
                w2col = wp.tile([P, 1], fdt, tag="w2col", bufs=8)
                nc.scalar.activation(
                    wsq[:csz, :], w_f32[:csz, :], AF.Square,
                    accum_out=w2col[:csz, :],
                )
                nc.gpsimd.dma_start(out=w2d[c0 : c0 + csz, :], in_=w2col[:csz, :])

                def put(kg, src):
                    # all w evacs on DVE: ScalarE front work is then just
                    # the 8 Squares (drained by ~30us), which unblocks the
                    # deferred x0..2 norms and through them the x5..7
                    # loads (x_f32 WAR chain).
                    dst = wT[:, kg * KG : (kg + 1) * KG, c0 : c0 + P]
                    if kg == 0:
                        nc.scalar.activation(dst, src, AF.Identity)
                    else:
                        nc.vector.tensor_copy(dst, src)
                transpose_evac(w_f32, put, identf, fdt)

            inv_sqrt_f = 1.0 / math.sqrt(F)

            def x_norm(i, x_f32):
                # Square scratch shares the w-phase wsq buffers (the w
                # squares are done before these run) to stay inside SBUF.
                xsq = wp.tile([P, F], bdt, tag="wsq", bufs=2)
                x2c = xp.tile([P, 1], fdt, tag="x2c", bufs=4)
                # accum_out = sum_f (x/sqrt(F))^2 = ||x_b||^2 / F
                nc.scalar.activation(
                    xsq[:, :], x_f32[:, :], AF.Square,
                    scale=inv_sqrt_f, accum_out=x2c[:, :],
                )
                nc.vector.tensor_scalar_mul(negx2s[i][:, :], x2c[:, :], -1.0)

            def x_prep(i, x_f32, n_act_evacs=1, with_norm=True):
                if with_norm:
                    x_norm(i, x_f32)
                # bf16 cast hop: x transposes run in phase G where the PE
                # is the bottleneck, and bf16 transposes stream 2x faster
                # than the 2-pass fp32 mode (55ns vs 110ns warm).
                x_bf = xp.tile([P, F], bdt, tag="x_bf", bufs=2)
                nc.vector.tensor_copy(x_bf[:, :], x_f32[:, :])

                def put(kg, src):
                    dst = xTs[i][:, kg * KG : (kg + 1) * KG, :]
                    if kg < n_act_evacs:
                        nc.scalar.activation(dst, src, AF.Identity)
                    else:
                        nc.vector.tensor_copy(dst, src)
                transpose_evac(x_bf, put, identb, bdt)

            def gemm(i):
                b0 = i * P
                xT = xTs[i]
                ps = pp.tile([P, 2 * 512], fdt, tag="ps")
                for n0, nsz in ((0, 512), (512, 488)):
                    for d in range(KD):
                        nc.tensor.matmul(
                            ps[:, n0 : n0 + nsz],
                            xT[:, 2 * d : 2 * d + 2, :],
                            wT[:, 2 * d : 2 * d + 2, n0 : n0 + nsz],
                            start=(d == 0), stop=(d == KD - 1),
                            perf_mode=DR,
                        )

                o_sb = ep.tile([P, C], fdt, tag="o_sb")
                nc.scalar.activation(
                    o_sb[:, 0:C], ps[:, 0:C], AF.Identity,
                    bias=negx2s[i][:, 0:1], scale=2.0 / F,
                )
                # w2 row-add on the otherwise-idle GpSimd: DVE carries the
                # cast + 3 evacs per round and would bind with this too.
                nc.gpsimd.tensor_add(o_sb[:, 0:C], o_sb[:, 0:C], w2bc[:, 0:C])
                nc.gpsimd.dma_start(out=out[b0 : b0 + P, :], in_=o_sb[:, :])

            # ---- Phase W: process all w tiles first, THEN the x0..LAG-1
            # preps.  The x tiles land on the sync ring AFTER w4..7, so
            # interleaving their transposes into the w loop stalled the
            # PE FIFO (x_k's transpose waited data while ready w tiles
            # queued behind it).  Post-loop, the w chain never blocks,
            # and the early x consumers run by ~45us, which also unblocks
            # the x_f32-pool WAR waits gating the x6+ loads. ----
            for j in range(CT):
                w_setup(j)
            for j in range(LAG):
                x_prep(j, xfs.pop(j), n_act_evacs=1, with_norm=True)
            # w2 gather: DRAM round-trip turns the per-tile columns into a
            # row replicated across all 128 partitions (stride-0 partition
            # AP on the DRAM side), then one ScalarE mul scales to
            # -||w_c||^2/F.  Nothing here touches the PE queue, so gemm(0)
            # fires as soon as wT/xT0 are resident; only the per-round
            # output ADD (DVE) waits on this chain.
            w2raw = constp.tile([P, CP], fdt)
            nc.gpsimd.dma_start(
                out=w2raw[:, 0:C],
                in_=w2d[0:C, :].rearrange("c one -> one c").partition_broadcast(P),
            )
            w2bc = constp.tile([P, CP], fdt)
            nc.scalar.mul(w2bc[:, 0:C], w2raw[:, 0:C], -1.0 / F)

            # ---- Phase G: 16 rounds of gemm(r) + x_prep(r+LAG) ----
            for r in range(BT):
                if r + LAG < BT:
                    xfs[r + LAG] = x_load(r + LAG)
                gemm(r)
                if r + LAG < BT:
                    x_prep(r + LAG, xfs.pop(r + LAG), n_act_evacs=1)

    return nc


def kernel(**inputs: np.ndarray) -> np.ndarray:
    global LAST_RESULTS
    x = np.ascontiguousarray(np.asarray(inputs["x"], dtype=np.float32))
    w = np.ascontiguousarray(np.asarray(inputs["w"], dtype=np.float32))
    assert x.shape == (B_TOTAL, F), x.shape
    assert w.shape == (C, F), w.shape

    from concourse.bass_utils import run_bass_kernel_spmd

    if "nc" not in _cache:
        nc = _build()
        _legalize_waits(nc)
        _cache["nc"] = nc
    nc = _cache["nc"]

    bs = B_TOTAL // N_CORES
    in_maps = [
        {"x": x[i * bs : (i + 1) * bs], "w": w} for i in range(N_CORES)
    ]
    res = run_bass_kernel_spmd(
        nc, in_maps, core_ids=list(range(N_CORES)),
        trace=bool(os.environ.get("BASS_TRACE")),
    )
    LAST_RESULTS = res
    return np.concatenate([r["out"] for r in res.results], axis=0)


if __name__ == "__main__":
    rng = np.random.default_rng(0)
    xs = rng.standard_normal((B_TOTAL, F), dtype=np.float32)
    ws = rng.standard_normal((C, F), dtype=np.float32) * math.sqrt(2.0 / F)
    o = kernel(x=xs, w=ws)
    print(o.shape, o.dtype, o[:2, :4])


# revision 72
# speedup vs baseline: 1.1043x; 1.1043x over previous
"""Trainium2 Bass kernel for nn_Euclidean (retrieval_knn).

Computes out[b, c] = -mean_f (x[b, f] - w[c, f])^2 for x [16384, 2048] f32,
w [1000, 2048] f32, via the algebraic expansion

    out = (2/F) * (x @ w.T  -  ||w_c||^2/2)  -  ||x_b||^2 / F

Sharding: data-parallel over the batch dim across 8 NeuronCores; w replicated.

Schedule: software pipeline with prep running LAG rounds ahead; phase G
round r issues [x_load(r+LAG), gemm(r), x_prep(r+LAG)] so the PE stays
HAM-warm (warm transposes ~110ns vs 210ns cold; DR matmuls 215ns).

  - All DMA issues precede any compute instruction.  w loads split
    across BOTH HWDGE rings (w0-3 scalar, w4-7 sync); x loads queue
    behind w4-7 on the sync ring FIFO so w gets full DMA bandwidth
    first.  gemm output stores + the tiny w2 column stores ride the
    GpSimd SWDGE ring so they never head-of-line-block an HWDGE queue.
  - Per-tile prep: fp32 tiles are transposed DIRECTLY on the PE (cayman
    transpose_mode, 2-pass fp32) -- no bf16 cast hop -- in 4 one-bank
    PSUM groups, evacuated to fp8e4 SBUF (w: 1 ScalarE + 3 DVE evacs;
    x: per-phase split).
  - Norms: ScalarE Square with fp32 accum.  w2col buffers are one-per-
    tile (the SWDGE store completion is ~5-8us; fewer buffers made each
    Square WAR-wait a store and serialized the front).  The x0..LAG-1
    preps (norms included) run AFTER the whole w loop: their tiles land
    behind w4..7 on the sync ring, so interleaving them into the w loop
    stalled ready w transposes behind them on the PE FIFO.
  - ||w_c||^2 reaches all partitions via DRAM round-trip: per-tile
    columns -> w2d, then one stride-0-partition broadcast DMA load and
    a ScalarE scale to -w2/F.  Nothing in that chain touches the PE
    queue, so gemm(0) fires as soon as wT/xT0 are resident; only the
    per-round DVE output ADD waits on it.

GEMM: 8 DoubleRow fp8 matmuls per 512-col half accumulate x @ w.T into
one [128,1024] PSUM tile (two banks).  lhsT/rhs are 3D APs [128 ki, 2
plane, free] whose planes are adjacent 128-k chunks (plane stride %16==0
satisfies s3_lw_dual_fp8_restrictions); each instruction contracts 256 k
with its LDWEIGHTS hidden under the previous matmul's stream.  The fp8
quantization noise only touches the (2/F)*x@w.T term (~1e-3 of the
output magnitude).  One ScalarE Identity evacuates both PSUM banks with
scale=2/F and per-partition bias -||x_b||^2/F, then DVE adds the
broadcast -||w_c||^2/F row and the result streams out via SWDGE.

Walrus encodes at most one semaphore wait per LDWEIGHTS/MM struct:
_legalize_waits splits any multi-wait instructions into standalone
EventSemaphore carriers.
"""

import math
import os
import sys

import numpy as np

if "/opt/trn_rl_repo" not in sys.path:
    sys.path.insert(0, "/opt/trn_rl_repo")

N_CORES = 8
B_TOTAL = 16384
F = 2048
C = 1000

_cache = {}
LAST_RESULTS = None


def _legalize_waits(nc):
    """Walrus encodes at most ONE sync-wait per instruction struct, but Tile's
    sem assignment freely attaches several. Split: hoist all but the last wait
    onto standalone EventSemaphore instructions (pure sem-op carriers) placed
    immediately before the over-limit instruction on the same engine queue."""
    import bass_rust
    import concourse.mybir as mybir

    n = 0
    for f in nc.m.functions:
        for bb in f.blocks:
            newlist = []
            for inst in bb.instructions:
                si = inst.sync_info
                if si is not None and len(si.on_wait) > 1:
                    waits = list(si.on_wait)
                    for w in waits[:-1]:
                        ev = mybir.InstEventSemaphore(
                            name=f"waitsplit_{n}", ins=[], outs=[]
                        )
                        ev.engine = inst.engine
                        ev.sync_info = bass_rust.SyncInfo(on_wait=[w], on_update=[])
                        newlist.append(ev)
                        n += 1
                    inst.sync_info = bass_rust.SyncInfo(
                        on_wait=[waits[-1]], on_update=list(si.on_update)
                    )
                newlist.append(inst)
            bb.instructions = newlist
    return n


def _build():
    import concourse.bass as bass
    import concourse.mybir as mybir
    from bass_rust import add_dep_helper
    from concourse.masks import make_identity
    from concourse.tile import TileContext

    P = 128
    KT = F // P                 # 16 contraction chunks of 128
    KD = KT // 2                # 8 DoubleRow plane-pairs of 256
    B = B_TOTAL // N_CORES      # 2048 batch rows per core
    BT = B // P                 # 16 batch chunks
    CP = 1024                   # padded class dim
    CT = CP // P                # 8 class chunks
    KG = 4                      # k-chunks per PSUM transpose group
    LAG = 4                     # x tiles prepped ahead of their gemm
    f8 = mybir.dt.float8e4
    bdt = mybir.dt.bfloat16
    fdt = mybir.dt.float32
    AF = mybir.ActivationFunctionType
    ALU = mybir.AluOpType
    DR = mybir.MatmulPerfMode.DoubleRow

    nc = bass.Bass()
    x = nc.dram_tensor("x", [B, F], fdt, kind="ExternalInput")
    w = nc.dram_tensor("w", [C, F], fdt, kind="ExternalInput")
    out = nc.dram_tensor("out", [B, C], fdt, kind="ExternalOutput")

    with TileContext(nc) as tc:
        with (
            tc.tile_pool(name="consts", bufs=1) as constp,
            tc.tile_pool(name="wstage", bufs=3) as wp,
            tc.tile_pool(name="xstage", bufs=3) as xp,
            tc.tile_pool(name="evac", bufs=3) as ep,
            tc.tile_pool(name="dram", bufs=1, space="DRAM") as dp,
            tc.tile_pool(name="psum", bufs=2, space="PSUM") as pp,
        ):
            # ---- DMA issues first: w on both rings, x behind w on sync ----
            w_f32s = []
            for j in range(CT):
                c0 = j * P
                csz = min(P, C - c0)
                w_f32 = wp.tile([P, F], fdt, tag="w_f32", bufs=8,
                                name=f"w_f32_{j}")
                if csz < P:
                    # the transpose below reads the pad rows; zero them
                    # BEFORE the load so the DMA overwrites the real rows.
                    pad_base = (csz // 32) * 32
                    nc.vector.memset(w_f32[pad_base:P, :], 0.0)
                eng = nc.scalar if j < CT // 2 else nc.sync
                eng.dma_start(out=w_f32[:csz, :], in_=w[c0 : c0 + csz, :])
                w_f32s.append(w_f32)

            def x_load(i):
                x_f32 = xp.tile([P, F], fdt, tag="x_f32", bufs=6,
                                name=f"x_f32_{i}")
                nc.sync.dma_start(out=x_f32[:, :], in_=x[i * P : (i + 1) * P, :])
                return x_f32

            xfs = {}
            for j in range(LAG):
                xfs[j] = x_load(j)

            # ---- constants (emitted after the issues; run during the DMA) --
            # Two identities: w tiles transpose straight from fp32 (the
            # PE has slack in the front and fp32 transpose_mode is a
            # 2-pass ~110ns/chunk affair), x tiles go through a DVE bf16
            # cast and 55ns/chunk bf16 transposes (PE is the bottleneck
            # in phase G).
            identf = constp.tile([P, P], fdt)
            make_identity(nc, identf[:, :])
            identb = constp.tile([P, P], bdt)
            make_identity(nc, identb[:, :])
            pwarm = pp.tile([P, P], fdt, tag="pst", bufs=4)
            nc.tensor.transpose(pwarm[:, :], identf[:, :], identf[:, :])

            # preload both ACT table sets (Square + Identity) off-path
            tw0 = wp.tile([1, 1], fdt, tag="actwarm0")
            tw1 = wp.tile([1, 1], fdt, tag="actwarm1")
            nc.vector.memset(tw0[:, :], 0.0)
            nc.scalar.activation(tw1[:, :], tw0[:, :], AF.Square)
            nc.scalar.activation(tw1[:, :], tw0[:, :], AF.Identity)

            wT = constp.tile([P, KT, CP], f8)     # w^T fp8, resident all kernel
            w2neg = constp.tile([1, CP], fdt)     # -||w_c||^2 / F
            w2row = constp.tile([1, CP], fdt)
            w2d = dp.tile([CP, 1], fdt)
            xTs = [
                constp.tile([P, KT, P], f8, name=f"xT_{i}") for i in range(BT)
            ]
            negx2s = [
                constp.tile([P, 1], fdt, name=f"negx2_{i}") for i in range(BT)
            ]

            dum_pool = {"prev": None}

            def transpose_evac(tile, put_evac, idn, pdt):
                """16 transposes in 4 one-bank PSUM groups + evacs.
                Multi-wait first-transposes are split by _legalize_waits."""
                for kg in range(KT // KG):
                    pst = pp.tile([P, KG * P], pdt, tag="pst", bufs=4)
                    for q in range(KG):
                        k = kg * KG + q
                        nc.tensor.transpose(
                            pst[:, q * P : (q + 1) * P],
                            tile[:, k * P : (k + 1) * P],
                            idn[:, :],
                        )
                    put_evac(kg, pst[:, :].rearrange("p (k c) -> p k c", k=KG))

            def w_setup(j):
                c0 = j * P
                csz = min(P, C - c0)
                w_f32 = w_f32s[j]
                # norm first: it gates the bias-rider fill and reads
                # w_f32 directly, so it never waits on the cast chain.
                wsq = wp.tile([P, F], bdt, tag="wsq", bufs=2)
                # one buffer per tile: the SWDGE store's completion is
                # slow (~5-8us); bufs=2 made each Square WAR-wait the
                # store from two tiles earlier, serializing the front.
                w2col = wp.tile([P, 1], fdt, tag="w2col", bufs=8)
                nc.scalar.activation(
                    wsq[:csz, :], w_f32[:csz, :], AF.Square,
                    accum_out=w2col[:csz, :],
                )
                nc.gpsimd.dma_start(out=w2d[c0 : c0 + csz, :], in_=w2col[:csz, :])

                def put(kg, src):
                    # all w evacs on DVE: ScalarE front work is then just
                    # the 8 Squares (drained by ~30us), which unblocks the
                    # deferred x0..2 norms and through them the x5..7
                    # loads (x_f32 WAR chain).
                    dst = wT[:, kg * KG : (kg + 1) * KG, c0 : c0 + P]
                    if kg == 0:
                        nc.scalar.activation(dst, src, AF.Identity)
                    else:
                        nc.vector.tensor_copy(dst, src)
                transpose_evac(w_f32, put, identf, fdt)

            inv_sqrt_f = 1.0 / math.sqrt(F)

            def x_norm(i, x_f32):
                # Square scratch shares the w-phase wsq buffers (the w
                # squares are done before these run) to stay inside SBUF.
                xsq = wp.tile([P, F], bdt, tag="wsq", bufs=2)
                x2c = xp.tile([P, 1], fdt, tag="x2c", bufs=4)
                # accum_out = sum_f (x/sqrt(F))^2 = ||x_b||^2 / F
                nc.scalar.activation(
                    xsq[:, :], x_f32[:, :], AF.Square,
                    scale=inv_sqrt_f, accum_out=x2c[:, :],
                )
                nc.vector.tensor_scalar_mul(negx2s[i][:, :], x2c[:, :], -1.0)

            def x_prep(i, x_f32, n_act_evacs=1, with_norm=True):
                if with_norm:
                    x_norm(i, x_f32)
                # bf16 cast hop: x transposes run in phase G where the PE
                # is the bottleneck, and bf16 transposes stream 2x faster
                # than the 2-pass fp32 mode (55ns vs 110ns warm).
                x_bf = xp.tile([P, F], bdt, tag="x_bf", bufs=2)
                nc.vector.tensor_copy(x_bf[:, :], x_f32[:, :])

                def put(kg, src):
                    dst = xTs[i][:, kg * KG : (kg + 1) * KG, :]
                    if kg < n_act_evacs:
                        nc.scalar.activation(dst, src, AF.Identity)
                    else:
                        nc.vector.tensor_copy(dst, src)
                transpose_evac(x_bf, put, identb, bdt)

            def gemm(i):
                b0 = i * P
                xT = xTs[i]
                ps = pp.tile([P, 2 * 512], fdt, tag="ps")
                for n0, nsz in ((0, 512), (512, 488)):
                    for d in range(KD):
                        nc.tensor.matmul(
                            ps[:, n0 : n0 + nsz],
                            xT[:, 2 * d : 2 * d + 2, :],
                            wT[:, 2 * d : 2 * d + 2, n0 : n0 + nsz],
                            start=(d == 0), stop=(d == KD - 1),
                            perf_mode=DR,
                        )

                o_sb = ep.tile([P, C], fdt, tag="o_sb")
                nc.scalar.activation(
                    o_sb[:, 0:C], ps[:, 0:C], AF.Identity,
                    bias=negx2s[i][:, 0:1], scale=2.0 / F,
                )
                if i < BT - LAG:
                    # w2 row-add on the otherwise-idle GpSimd: DVE carries
                    # the cast + 3 evacs per round and would bind too.
                    nc.gpsimd.tensor_add(o_sb[:, 0:C], o_sb[:, 0:C], w2bc[:, 0:C])
                    nc.gpsimd.dma_start(out=out[b0 : b0 + P, :], in_=o_sb[:, :])
                else:
                    # tail rounds have no prep work: DVE (1.2us vs 2.8)
                    # and the idle sync HWDGE ring shorten the exit drain.
                    nc.vector.tensor_add(o_sb[:, 0:C], o_sb[:, 0:C], w2bc[:, 0:C])
                    nc.sync.dma_start(out=out[b0 : b0 + P, :], in_=o_sb[:, :])

            # ---- Phase W: process all w tiles first, THEN the x0..LAG-1
            # preps.  The x tiles land on the sync ring AFTER w4..7, so
            # interleaving their transposes into the w loop stalled the
            # PE FIFO (x_k's transpose waited data while ready w tiles
            # queued behind it).  Post-loop, the w chain never blocks,
            # and the early x consumers run by ~45us, which also unblocks
            # the x_f32-pool WAR waits gating the x6+ loads. ----
            for j in range(CT):
                w_setup(j)
            for j in range(LAG):
                x_prep(j, xfs.pop(j), n_act_evacs=1, with_norm=True)
            # w2 gather: DRAM round-trip turns the per-tile columns into a
            # row replicated across all 128 partitions (stride-0 partition
            # AP on the DRAM side), then one ScalarE mul scales to
            # -||w_c||^2/F.  Nothing here touches the PE queue, so gemm(0)
            # fires as soon as wT/xT0 are resident; only the per-round
            # output ADD (DVE) waits on this chain.
            w2raw = constp.tile([P, CP], fdt)
            nc.gpsimd.dma_start(
                out=w2raw[:, 0:C],
                in_=w2d[0:C, :].rearrange("c one -> one c").partition_broadcast(P),
            )
            w2bc = constp.tile([P, CP], fdt)
            nc.scalar.mul(w2bc[:, 0:C], w2raw[:, 0:C], -1.0 / F)

            # ---- Phase G: 16 rounds of gemm(r) + x_prep(r+LAG) ----
            for r in range(BT):
                if r + LAG < BT:
                    xfs[r + LAG] = x_load(r + LAG)
                gemm(r)
                if r + LAG < BT:
                    x_prep(r + LAG, xfs.pop(r + LAG), n_act_evacs=1)

    return nc


def kernel(**inputs: np.ndarray) -> np.ndarray:
    global LAST_RESULTS
    x = np.ascontiguousarray(np.asarray(inputs["x"], dtype=np.float32))
    w = np.ascontiguousarray(np.asarray(inputs["w"], dtype=np.float32))
    assert x.shape == (B_TOTAL, F), x.shape
    assert w.shape == (C, F), w.shape

    from concourse.bass_utils import run_bass_kernel_spmd

    if "nc" not in _cache:
        nc = _build()
        _legalize_waits(nc)
        _cache["nc"] = nc
    nc = _cache["nc"]

    bs = B_TOTAL // N_CORES
    in_maps = [
        {"x": x[i * bs : (i + 1) * bs], "w": w} for i in range(N_CORES)
    ]
    res = run_bass_kernel_spmd(
        nc, in_maps, core_ids=list(range(N_CORES)),
        trace=bool(os.environ.get("BASS_TRACE")),
    )
    LAST_RESULTS = res
    return np.concatenate([r["out"] for r in res.results], axis=0)


if __name__ == "__main__":
    rng = np.random.default_rng(0)
    xs = rng.standard_normal((B_TOTAL, F), dtype=np.float32)
    ws = rng.standard_normal((C, F), dtype=np.float32) * math.sqrt(2.0 / F)
    o = kernel(x=xs, w=ws)
    print(o.shape, o.dtype, o[:2, :4])
